# revision 63
# baseline (speedup 1.0000x reference)
"""Trainium2 Bass kernel for a dense transformer block (B=4, T=2048, C=1024, 16 heads).

Sharding over 8 NeuronCores: core i handles batch b=i//2 with shard s=i%2.
 - fused front end, pipelined per 512-token chunk j: LN1 (bn_stats, 4-wide
   scalar chains) + QKV GEMMs for chunk j overlap the exp/AV stream of the
   causal attention for chunk j's queries (8 local heads, c-slice
   [512s, 512s+512)); proj pass A is interleaved into the j=3 stream
 - peer-half attention outputs exchanged via 2 merged ReduceScatters over
   core pairs (fp8 payload, zero-masked slots, fully SPMD-symmetric)
 - proj (local/remote passes) + LN2 + FFN + residuals on the core's t-half,
   with the second half's proj/LN2 interleaved into the first half's FFN

Precision: QKV / AV / proj GEMMs in single-quant fp8e4 DoubleRow; FFN in
hi/lo-split 3-term fp8e4 DoubleRow (W*a = Wh*ah + Wh*al + Wl*ah), which is
more accurate than bf16 at 0.75x the bf16 PE cost; QK in fp8-operand
single-rate matmuls; accumulation fp32 in PSUM. Weight fp8 scales (x32/x64)
are folded into post-GEMM activation scales and host-side masks. LayerNorm
gain/bias and the attention 1/sqrt(C) factor are folded into weights on the
host. tri-mask multiplies, diagonal-pair memsets, denominator partition-
broadcasts and bulk prefetch DMAs run on the otherwise-idle GpSimd engine.
"""

from contextlib import ExitStack

import ml_dtypes
import numpy as np

import concourse.bass as bass
import concourse.mybir as mybir
import concourse.tile as tile
from concourse import bacc
from concourse.bass_utils import run_bass_kernel_spmd

f32 = mybir.dt.float32
f32r = mybir.dt.float32r
bf16 = mybir.dt.bfloat16
fp8 = mybir.dt.float8e4
DR = mybir.MatmulPerfMode.DoubleRow
E4 = ml_dtypes.float8_e4m3
W1S = 32.0           # host scale on W1/Wq/Wk/Wv/Wo (fp8 dynamic range)
W2S = 64.0           # host scale on W2
FFS = 1.0 / (W1S * W2S)
AF = mybir.ActivationFunctionType
ALU = mybir.AluOpType
AX = mybir.AxisListType

B, T, C = 4, 2048, 1024
NH, D = 16, 64
F = 4 * C
H = C // 2            # per-core head c-slice (8 heads)
TH = T // 2           # per-core t-half for proj/FFN
EPS = 1e-5
RG = [[0, 1], [2, 3], [4, 5], [6, 7]]

_CACHE = {}


class S:
    """Shared build state."""
    pass


def _ln_stats(nc, st, xts, tag):
    """LayerNorm stats for up to 4 tiles [128, C] at once; returns (rsig, nmu)
    [128, n] tiles: one small-op chain instead of one per tile."""
    work = st.work
    n = len(xts)
    stats = work.tile([128, n, 12], f32, name="bnst", tag=f"bnst{tag}")
    for i, xt in enumerate(xts):
        nc.vector.bn_stats(stats[:, i, 0:6], xt[:, 0:512])
        nc.vector.bn_stats(stats[:, i, 6:12], xt[:, 512:1024])
    agg = work.tile([128, n, 2], f32, name="bnagg", tag=f"bnagg{tag}")
    for i in range(n):
        nc.vector.bn_aggr(agg[:, i, :], stats[:, i, :])
    var = work.tile([128, n], f32, name="var", tag=f"var{tag}")
    nc.vector.tensor_scalar_add(var[:], agg[:, :, 1], EPS)
    sd = work.tile([128, n], f32, name="sd", tag=f"sd{tag}")
    nc.scalar.activation(sd[:], var[:], AF.Sqrt)
    rsig = work.tile([128, n], f32, name="rsig", tag=f"rsig{tag}")
    with nc.allow_low_precision(reason="LN rsqrt"):
        nc.vector.reciprocal(rsig[:], sd[:])
    nmu = work.tile([128, n], f32, name="nmu", tag=f"nmu{tag}")
    nc.vector.tensor_tensor(nmu[:], agg[:, :, 0], rsig[:], ALU.mult)
    nc.vector.tensor_scalar_mul(nmu[:], nmu[:], -1.0)
    return rsig, nmu


def _ln_apply(nc, xt, dst, rsig, nmu, i):
    nc.scalar.activation(dst[:], xt[:], AF.Identity,
                         bias=nmu[:, i:i + 1], scale=rsig[:, i:i + 1])


def _front_setup(nc, st):
    """Pools, weight prefetch, persistent tiles and helpers for the fused
    QKV+attention front end."""
    st.qkvp = st.tc.tile_pool(name="qkv", bufs=1)
    qkv = st.qkvp.__enter__()
    st.wqkvp = st.tc.tile_pool(name="wqkv", bufs=1)
    wqkv = st.wqkvp.__enter__()
    st.xhp = st.tc.tile_pool(name="xh", bufs=2)
    st.xh = st.xhp.__enter__()
    st.htcp = st.tc.tile_pool(name="htc", bufs=2)
    st.htc_pool = st.htcp.__enter__()
    st.psf_p = st.tc.tile_pool(name="psfront", bufs=2, space="PSUM")
    st.psf = st.psf_p.__enter__()
    st.ps_po_p = st.tc.tile_pool(name="pspo", bufs=1, space="PSUM")
    st.ps_po = st.ps_po_p.__enter__()

    st.w1pre_p = st.tc.tile_pool(name="w1pre", bufs=1, side="right")
    w1pre = st.w1pre_p.__enter__()
    st.wop = st.tc.tile_pool(name="wop", bufs=1, side="right")
    wop = st.wop.__enter__()
    st.attp = st.tc.tile_pool(name="attp", bufs=1, side="right")
    attp = st.attp.__enter__()
    st.attsbp = st.tc.tile_pool(name="attsb", bufs=1, side="right")
    attsb = st.attsbp.__enter__()
    st.xrpp = st.tc.tile_pool(name="xrp", bufs=1, side="right")
    xrp = st.xrpp.__enter__()
    st.aw_p = st.tc.tile_pool(name="aw", bufs=1)
    st.aw = st.aw_p.__enter__()

    # x chunk 0 first so LN can start immediately
    st.xts = []
    for tt in range(4):
        xt = st.xh.tile([128, C], f32, name="xt", tag=f"xt{tt % 4}")
        nc.gpsimd.dma_start(xt[:], st.x_h[tt * 128:(tt + 1) * 128, :])
        st.xts.append(xt)

    st.wq_sb = [wqkv.tile([128, 2, H], fp8, name=f"wq{k}", tag=f"wq{k}")
                for k in range(4)]
    st.wk_sb = [wqkv.tile([128, 2, H], fp8, name=f"wk{k}", tag=f"wk{k}")
                for k in range(4)]
    st.wv_sb = [wqkv.tile([128, 2, H], fp8, name=f"wv{k}", tag=f"wv{k}")
                for k in range(4)]
    for wsb, wh in ((st.wq_sb, st.wq_h), (st.wk_sb, st.wk_h),
                    (st.wv_sb, st.wv_h)):
        for kp in range(4):
            nc.sync.dma_start(
                wsb[kp][:], wh[kp * 256:(kp + 1) * 256, :]
                .rearrange("(kt p) h -> p kt h", p=128))
    for t_, h_ in [(st.bq_sb, st.bq_h), (st.bk_sb, st.bk_h),
                   (st.bv_sb, st.bv_h), (st.onesr, st.onesr_h),
                   (st.ones8, st.ones8_h),
                   (st.tri, st.tri_h), (st.sel_sb, st.sel_h),
                   (st.seln_sb, st.seln_h), (st.b1_sb, st.b1_h)]:
        nc.sync.dma_start(t_[:], h_[:])

    st.qT = [qkv.tile([128, T], fp8, name=f"qT{i}", tag=f"qT{i}")
             for i in range(4)]
    st.kT = [qkv.tile([128, T], fp8, name=f"kT{i}", tag=f"kT{i}")
             for i in range(4)]
    st.vn = [qkv.tile([128, 2, 8, 80], fp8, name=f"vn{i}", tag=f"vn{i}")
             for i in range(8)]

    # prefetches for later phases (independent of front compute)
    st.wo_sb = [wop.tile([128, 2, C], fp8, name=f"wo{i}", tag=f"wo{i}")
                for i in range(4)]
    st.xr = [xrp.tile([128, C], f32, name=f"xr{t}", tag=f"xr{t}")
             for t in range(8)]
    st.w1g0 = [w1pre.tile([128, 2, 8, 128], fp8, name=f"w1g0_{f}",
                          tag=f"w1g0_{f}") for f in range(4)]

    st.asb = [attsb.tile([128, 4, 512], fp8, name=f"asb{h}",
                         tag=f"asb{h}") for h in range(2)]
    st.attA = [attp.tile([128, 2, T], fp8, name=f"attA{i}", tag=f"attA{i}")
               for i in range(2)]

    st.x2 = st.xr
    st.pending_tail = [None]

    def flush_tail():
        if st.pending_tail[0] is not None:
            st.pending_tail[0]()
            st.pending_tail[0] = None

    def emit_rs(parity):
        nc.gpsimd.collective_compute(
            "ReduceScatter", ALU.add, replica_groups=RG,
            ins=[st.rs_in[parity][:]], outs=[st.rs_out[parity][:]])
        nc.gpsimd.dma_start(st.asb[parity][:], st.rs_out[parity][:])

    st.flush_tail = flush_tail
    st.emit_rs = emit_rs


def _front_qkv_j(nc, st, j):
    """LN1 + QKV for t-chunk j."""
    htc = st.htc_pool.tile([128, 8, 4, 128], bf16, name="htc", tag="htc")
    htc8 = st.htc_pool.tile([128, 8, 4, 128], fp8, name="htc8", tag="htc8")
    xts = []
    for tt4 in range(4):  # t-tiles of 128 within the chunk
        tt = j * 4 + tt4
        if j == 0:
            xt = st.xts[tt4]
        else:
            xt = st.xh.tile([128, C], f32, name="xt", tag=f"xt{tt % 4}")
            nc.gpsimd.dma_start(xt[:], st.x_h[tt * 128:(tt + 1) * 128, :])
        xts.append(xt)
    rsig, nmu = _ln_stats(nc, st, xts, "a")
    for tt4 in range(4):
        ht = st.xh.tile([128, C], bf16, name="ht", tag=f"ht{tt4 % 2}")
        _ln_apply(nc, xts[tt4], ht, rsig, nmu, tt4)
        nc.sync.dma_start_transpose(htc[:, :, tt4, :], ht[:])
    nc.gpsimd.dma_start(htc8[:].rearrange("p a b c -> p (a b c)"),
                        htc[:].rearrange("p a b c -> p (a b c)"))
    for dst, wsb, bsb in ((st.qT, st.wq_sb, st.bq_sb),
                          (st.kT, st.wk_sb, st.bk_sb)):
        for co in range(4):
            pg = st.psf.tile([128, 512], f32, name="ps_qk", tag="qk")
            for kp in range(4):
                nc.tensor.matmul(pg[:],
                                 wsb[kp][:, :, co * 128:(co + 1) * 128],
                                 htc8[:, 2 * kp:2 * kp + 2, :, :],
                                 start=(kp == 0), stop=(kp == 3),
                                 perf_mode=DR)
            if st.qkv_bias:
                nc.scalar.activation(dst[co][:, j * 512:(j + 1) * 512], pg[:],
                                     AF.Identity, bias=bsb[:, co:co + 1],
                                     scale=1.0 / W1S)
            else:
                nc.vector.tensor_scalar_mul(
                    dst[co][:, j * 512:(j + 1) * 512], pg[:], 1.0 / W1S)
    for tt4 in range(4):
        tt = j * 4 + tt4
        pg = st.psf.tile([128, 512], f32, name="ps_v", tag="qk")
        for kp in range(4):
            nc.tensor.matmul(pg[:], htc8[:, 2 * kp:2 * kp + 2, tt4, :],
                             st.wv_sb[kp][:],
                             start=(kp == 0), stop=(not st.qkv_bias),
                             perf_mode=DR)
        vslot = st.vn[tt // 2][:, tt % 2]
        if st.qkv_bias:
            nc.tensor.matmul(pg[:], st.onesr[:, 0:128], st.bv_sb[:],
                             start=False, stop=True)
            nc.scalar.activation(
                vslot[:, :, 0:64],
                pg[:].rearrange("p (h d) -> p h d", h=8),
                AF.Identity, scale=1.0 / W1S)
        else:
            nc.vector.tensor_scalar_mul(
                vslot[:, :, 0:64],
                pg[:].rearrange("p (h d) -> p h d", h=8), 1.0 / W1S)
        nc.sync.dma_start(
            vslot[:, :, 64:65],
            st.ones8[:].rearrange("p (h o) -> p h o", h=8))

def _front_att_j(nc, st, j):
    """Causal attention for chunk j's queries over all keys <= j."""
    for hp in range(4):
        tq0 = j * 512
        nk = 4 * (j + 1)
        po = [st.ps_po.tile([128, 512], f32, tag="po0", name="po0"),
              st.ps_po.tile([128, 512], f32, tag="po1", name="po1")]

        def emit_qk(kk):
            r = 128 * (kk - 4 * j) if kk >= 4 * j else 0
            pqk = st.psf.tile([128, 1024], f32, tag="qkp", name="qkp")
            for bi, b0 in enumerate((0, 64)):
                nc.tensor.matmul(
                    pqk[:, bi * 512 + r:bi * 512 + 512],
                    st.kT[hp][b0:b0 + 64, kk * 128:(kk + 1) * 128],
                    st.qT[hp][b0:b0 + 64, tq0 + r:tq0 + 512],
                    start=True, stop=True)
            return pqk

        pqk_next = emit_qk(0)
        ptbp = None
        r_lo = 0
        for kk in range(nk):
            r = 128 * (kk - 4 * j) if kk >= 4 * j else 0
            pqk = pqk_next
            if kk + 1 < nk:
                pqk_next = emit_qk(kk + 1)
            if kk % 2 == 0:
                ptbp = st.ptp.tile([128, 2, 1024], fp8, name="ptbp", tag="pt")
                r_lo = r
            ptb = ptbp[:, kk % 2, :]
            esc = 1.0
            if r == 0:
                nc.scalar.activation(ptb, pqk[:], AF.Exp, scale=esc)
            else:
                nc.scalar.activation(
                    ptb.rearrange("p (b w) -> p b w", b=2)[:, :, r:512],
                    pqk[:].rearrange("p (b w) -> p b w", b=2)[:, :, r:512],
                    AF.Exp, scale=esc)
            if kk == 0:
                st.flush_tail()
                if j == 3 and hp == 0:
                    st.emit_rs(0)
            if kk >= 4 * j:
                nc.gpsimd.tensor_tensor(
                    ptb.rearrange("p (b w) -> p b w", b=2)[:, :, r:r + 128],
                    ptb.rearrange("p (b w) -> p b w", b=2)[:, :, r:r + 128],
                    st.tri[:, None, :].to_broadcast((128, 2, 128)),
                    ALU.mult)
            if kk % 2 == 1:
                if r > r_lo:
                    # zero the odd half's columns [r_lo, r) (never written)
                    nc.gpsimd.memset(
                        ptbp[:, 1, :].rearrange(
                            "p (b w) -> p b w", b=2)[:, :, r_lo:r], 0)
                for bi in range(2):
                    h = 2 * hp + bi
                    nc.tensor.matmul(
                        po[bi][0:65, r_lo:512],
                        st.vn[kk // 2][:, :, h, 0:65],
                        ptbp[:, :, bi * 512 + r_lo:bi * 512 + 512],
                        start=(kk == 1), stop=(kk == nk - 1),
                        perf_mode=DR)
        sj = j // 2
        # softmax tail, deferred past the next block's first QK so the PE
        # queue is not blocked waiting on the reciprocal chain.
        def make_tail(hp=hp, j=j, sj=sj, tq0=tq0, po=po):
            def tail():
                for bi, b0 in enumerate((0, 64)):
                    dn = st.aw.tile([1, 512], f32, name="dn", tag=f"dn{bi}")
                    nc.vector.tensor_copy(out=dn[:], in_=po[bi][64:65, :])
                    rrow = st.aw.tile([1, 512], f32, name="rrow",
                                      tag=f"rrow{bi}")
                    with nc.allow_low_precision(reason="softmax denom"):
                        nc.vector.reciprocal_approx_fast(rrow[:], dn[:])
                    rbi = st.aw.tile([64, 512], f32, name="rbi",
                                     tag=f"rbi{bi}")
                    nc.gpsimd.partition_broadcast(rbi[:], rrow[:])
                    if _CACHE.get("debug") and j == 0 and hp == 0 and bi == 0:
                        nc.sync.dma_start(st.drbi_h[:], rbi[:])
                    nc.vector.scalar_tensor_tensor(
                        st.attA[hp // 2][b0:b0 + 64, hp % 2, tq0:tq0 + 512],
                        po[bi][0:64, :], st.sel_sb[0:64, sj:sj + 1], rbi[:],
                        ALU.mult, ALU.mult)
                    attBc = st.aw.tile([64, 512], fp8, name="attBc",
                                       tag=f"attBc{bi}")
                    nc.vector.scalar_tensor_tensor(
                        attBc[:], po[bi][0:64, :],
                        st.seln_sb[0:64, sj:sj + 1], rbi[:],
                        ALU.mult, ALU.mult)
                    nc.sync.dma_start(
                        st.rs_in[j % 2][sj, b0:b0 + 64, hp, :],
                        attBc[:])
            return tail
        st.pending_tail[0] = make_tail()
        if j == 3:
            if hp == 3:
                st.flush_tail()
                st.emit_rs(1)
            _pass_a_tt(nc, st, hp, st.psf, "qk")

def _front_prefetch_j(nc, st, j):
    # distributed prefetches for the proj/FFN phases
    if j == 1:
        for i in range(4):
            nc.sync.dma_start(st.wo_sb[i][:],
                              st.wo_h[i * 256:(i + 1) * 256, :]
                              .rearrange("(kt p) c -> p kt c", p=128))
        nc.sync.dma_start(st.b2_sb[:], st.b2_h[:])
    elif j == 2:
        for tt in range(8):
            nc.sync.dma_start(st.xr[tt][:],
                              st.xres_h[tt * 128:(tt + 1) * 128, :])
    elif j == 3:
        for f in range(4):
            nc.sync.dma_start(st.w1g0[f][:],
                              st.w1_h[f].rearrange("h p c -> p h c"))


def _pass_a_tt(nc, st, tt, pool, tag):
    for cc in range(2):
        pg = pool.tile([128, 512], f32, tag=tag, name="pja")
        for i in range(2):
            for half in range(2):
                nc.tensor.matmul(
                    pg[:],
                    st.attA[i][:, :, half * TH + tt * 128:
                               half * TH + (tt + 1) * 128],
                    st.wo_sb[i][:, :, cc * 512:(cc + 1) * 512],
                    start=(i == 0 and half == 0),
                    stop=(i == 1 and half == 1), perf_mode=DR)
        nc.vector.scalar_tensor_tensor(
            st.xr[tt][:, cc * 512:(cc + 1) * 512], pg[:], 1.0 / W1S,
            st.xr[tt][:, cc * 512:(cc + 1) * 512], ALU.mult, ALU.add)


def _front_finish(nc, st):
    st.aw_p.__exit__(None, None, None)
    st.ps_po_p.__exit__(None, None, None)
    st.psf_p.__exit__(None, None, None)
    st.htcp.__exit__(None, None, None)
    st.xhp.__exit__(None, None, None)
    st.wqkvp.__exit__(None, None, None)


def _phase_proj(nc, st):
    """Projection + residual, split into a local pass (runs during the last
    ReduceScatter) and a remote pass. Fills st.x2."""
    st.qkvp.__exit__(None, None, None)
    ps_pj_p = st.tc.tile_pool(name="pspj", bufs=2, space="PSUM")
    ps_pj = ps_pj_p.__enter__()
    st.h2wp = st.tc.tile_pool(name="h2w", bufs=2)
    st.h2w = st.h2wp.__enter__()
    st.h2pp = st.tc.tile_pool(name="h2p", bufs=1)
    h2p = st.h2pp.__enter__()
    st.h2c = [h2p.tile([128, 8, 4, 128], bf16, name=f"h2c{i}", tag="h2c")
              for i in range(2)]
    st.h2hi = [h2p.tile([128, 8, 512], fp8, name=f"h2hi{i}", tag=f"h2hi{i}")
               for i in range(2)]
    st.h2lo = [h2p.tile([128, 8, 512], fp8, name=f"h2lo{i}", tag=f"h2lo{i}")
               for i in range(2)]
    st.ps_pj = ps_pj
    st.ps_pj_p = ps_pj_p
    # pass A rows 4-7 (0-3 were interleaved into the j=3 attention stream)
    for tt in range(4, 8):
        _pass_a_tt(nc, st, tt, ps_pj, "pj")
    _pass_b_half(nc, st, 0)
    if _CACHE.get("debug"):
        nc.sync.dma_start(st.dasb_h[:], st.asb[0][:].bitcast(f32))
        nc.sync.dma_start(st.datA_h[:], st.attA[0][:].bitcast(f32))
        nc.sync.dma_start(st.dx2_h[:], st.x2[0][:])


def _pass_b_half(nc, st, half, pool=None, tag="pj"):
    """Remote-head proj contributions + LN2 + transpose for one t-half."""
    if pool is None:
        pool = st.ps_pj
    for tt in range(4 * half, 4 * half + 4):
        for cc in range(2):
            pg = pool.tile([128, 512], f32, tag=tag, name="pj")
            for i in range(2):
                nc.tensor.matmul(
                    pg[:],
                    st.asb[half][:, 2 * i:2 * i + 2,
                                 (tt % 4) * 128:(tt % 4 + 1) * 128],
                    st.wo_sb[2 + i][:, :, cc * 512:(cc + 1) * 512],
                    start=(i == 0), stop=(i == 1), perf_mode=DR)
            nc.vector.scalar_tensor_tensor(
                st.x2[tt][:, cc * 512:(cc + 1) * 512], pg[:], 1.0 / W1S,
                st.x2[tt][:, cc * 512:(cc + 1) * 512], ALU.mult, ALU.add)
    rsig, nmu = _ln_stats(nc, st, [st.x2[t] for t in
                                   range(4 * half, 4 * half + 4)], "b")
    for tt in range(4 * half, 4 * half + 4):
        h2t = st.h2w.tile([128, C], bf16, name="h2t", tag=f"h2t{tt % 2}")
        _ln_apply(nc, st.x2[tt], h2t, rsig, nmu, tt % 4)
        nc.sync.dma_start_transpose(st.h2c[half][:, :, tt % 4, :], h2t[:])
        # per-tile hi/lo e4m3 split (transposed layout slice [p, 8, 1, 128])
        h2c_v = st.h2c[half][:, :, tt % 4, :]
        hi_v = st.h2hi[half][:].rearrange(
            "p k (a b) -> p k a b", a=4)[:, :, tt % 4, :]
        lo_v = st.h2lo[half][:].rearrange(
            "p k (a b) -> p k a b", a=4)[:, :, tt % 4, :]
        nc.scalar.copy(hi_v, h2c_v)
        nc.gpsimd.tensor_tensor(lo_v, h2c_v, hi_v, ALU.subtract)


def _open_ffn_pools(nc, st):
    st.ps_f1_p = st.tc.tile_pool(name="psf1", bufs=2, space="PSUM")
    st.ps_f1 = st.ps_f1_p.__enter__()
    st.ps_f2_p = st.tc.tile_pool(name="psf2", bufs=2, space="PSUM")
    st.ps_f2 = st.ps_f2_p.__enter__()
    st.yacp = st.tc.tile_pool(name="yac", bufs=1)
    yac = st.yacp.__enter__()
    st.w1pp = st.tc.tile_pool(name="w1p", bufs=4)
    st.w1p = st.w1pp.__enter__()
    st.w2pp = st.tc.tile_pool(name="w2p", bufs=2)
    st.w2p = st.w2pp.__enter__()
    st.utpp = st.tc.tile_pool(name="utp", bufs=2)
    st.utp = st.utpp.__enter__()
    st.y_acc = [yac.tile([128, C], f32, name=f"ya{t}", tag=f"ya{t % 4}")
                for t in range(8)]


def _phase_ffn_half(nc, st, tch):
    """FFN for one t-half: fp8 hi/lo 3-term DoubleRow GEMMs, grouped ff-dim
    accumulation, residual, output DMA."""
    for g in range(4):
        if tch == 0 and g == 3:
            # overlap the second half's proj/LN2 with this half's last group
            _pass_b_half(nc, st, 1)
        uth_g, utl_g = [], []
        for fp_ in range(4):        # f-tile pairs within the group
            uth = st.utp.tile([128, 2, 512], fp8, name="uth", tag=f"uth{fp_}")
            utl = st.utp.tile([128, 2, 512], fp8, name="utl", tag=f"utl{fp_}")
            for sub in range(2):
                f = g * 8 + fp_ * 2 + sub
                if g == 0 and fp_ * 2 + sub < 4:
                    w1c = st.w1g0[fp_ * 2 + sub]
                else:
                    w1c = st.w1p.tile([128, 2, 8, 128], fp8, name="w1c",
                                      tag="w1c")
                    nc.sync.dma_start(
                        w1c[:], st.w1_h[f].rearrange("h p c -> p h c"))
                pg = st.ps_f1.tile([128, 512], f32, tag="f1", name="f1")
                for kp in range(4):
                    for ti, (ws, hs) in enumerate(
                            ((0, st.h2hi[tch]), (0, st.h2lo[tch]),
                             (1, st.h2hi[tch]))):
                        nc.tensor.matmul(
                            pg[:], w1c[:, ws, 2 * kp:2 * kp + 2, :],
                            hs[:, 2 * kp:2 * kp + 2, :],
                            start=(kp == 0 and ti == 0),
                            stop=(kp == 3 and ti == 2), perf_mode=DR)
                nc.scalar.activation(uth[:, sub, :], pg[:], AF.Relu,
                                     bias=st.b1_sb[:, f:f + 1])
                nc.vector.scalar_tensor_tensor(
                    utl[:, sub, :], pg[:], 0.0, uth[:, sub, :],
                    ALU.max, ALU.subtract)
            uth_g.append(uth)
            utl_g.append(utl)
        for cc in range(2):
            cs_ = slice(cc * 512, (cc + 1) * 512)
            w2hi_g = st.w2p.tile([128, 8, 512], fp8, name="w2hi", tag="w2hi")
            w2lo_g = st.w2p.tile([128, 8, 512], fp8, name="w2lo", tag="w2lo")
            for hl, t_ in ((0, w2hi_g), (1, w2lo_g)):
                nc.sync.dma_start(
                    t_[:], st.w2_h[hl, 1024 * g:1024 * (g + 1), cs_]
                    .rearrange("(ft p) c -> p ft c", p=128))
            for tt in range(4 * tch, 4 * tch + 4):
                pg = st.ps_f2.tile([128, 512], f32, tag="f2", name="f2")
                ts_ = slice((tt % 4) * 128, (tt % 4 + 1) * 128)
                for i in range(4):
                    for ti, (lh, rh) in enumerate(
                            ((uth_g[i], w2hi_g), (uth_g[i], w2lo_g),
                             (utl_g[i], w2hi_g))):
                        nc.tensor.matmul(
                            pg[:], lh[:, :, ts_],
                            rh[:, 2 * i:2 * i + 2, :],
                            start=(i == 0 and ti == 0),
                            stop=(False if g == 0 else (i == 3 and ti == 2)),
                            perf_mode=DR)
                if g == 0:
                    nc.tensor.matmul(pg[:], st.onesr[:, 0:128],
                                     st.b2_sb[:, cs_],
                                     start=False, stop=True)
                    nc.vector.scalar_tensor_tensor(
                        st.y_acc[tt][:, cs_], pg[:], FFS,
                        st.x2[tt][:, cs_], ALU.mult, ALU.add)
                else:
                    nc.vector.scalar_tensor_tensor(
                        st.y_acc[tt][:, cs_], pg[:], FFS,
                        st.y_acc[tt][:, cs_], ALU.mult, ALU.add)
    for tt in range(4 * tch, 4 * tch + 4):
        nc.sync.dma_start(st.y_h[tt * 128:(tt + 1) * 128, :], st.y_acc[tt][:])


def _close_ffn_pools(nc, st):
    st.xrpp.__exit__(None, None, None)
    st.attsbp.__exit__(None, None, None)
    st.attp.__exit__(None, None, None)
    st.wop.__exit__(None, None, None)
    st.w1pre_p.__exit__(None, None, None)
    st.utpp.__exit__(None, None, None)
    st.w2pp.__exit__(None, None, None)
    st.w1pp.__exit__(None, None, None)
    st.yacp.__exit__(None, None, None)
    st.h2pp.__exit__(None, None, None)
    st.h2wp.__exit__(None, None, None)
    st.ps_f2_p.__exit__(None, None, None)
    st.ps_f1_p.__exit__(None, None, None)
    st.ps_pj_p.__exit__(None, None, None)


def build_program(qkv_bias=False):
    key = ("nc", qkv_bias)
    if key in _CACHE:
        return _CACHE[key]
    nc = bacc.Bacc(None)
    st = S()
    st.qkv_bias = qkv_bias

    st.x_h = nc.declare_dram_parameter("x", [T, C], f32, isOutput=False)
    st.xres_h = nc.declare_dram_parameter("xres", [TH, C], f32, isOutput=False)
    st.wq_h = nc.declare_dram_parameter("wq", [C, H], fp8, isOutput=False)
    st.wk_h = nc.declare_dram_parameter("wk", [C, H], fp8, isOutput=False)
    st.wv_h = nc.declare_dram_parameter("wv", [C, H], fp8, isOutput=False)
    st.bq_h = nc.declare_dram_parameter("bq", [128, 4], f32, isOutput=False)
    st.bk_h = nc.declare_dram_parameter("bk", [128, 4], f32, isOutput=False)
    st.bv_h = nc.declare_dram_parameter("bv", [1, H], bf16, isOutput=False)
    st.wo_h = nc.declare_dram_parameter("wo", [C, C], fp8, isOutput=False)
    st.w1_h = nc.declare_dram_parameter("w1", [32, 2, 128, 1024], fp8,
                                        isOutput=False)
    st.b1_h = nc.declare_dram_parameter("b1", [128, 32], f32, isOutput=False)
    st.w2_h = nc.declare_dram_parameter("w2", [2, F, C], fp8, isOutput=False)
    st.b2_h = nc.declare_dram_parameter("b2", [1, C], bf16, isOutput=False)
    st.tri_h = nc.declare_dram_parameter("tri", [128, 128], fp8, isOutput=False)
    st.onesr_h = nc.declare_dram_parameter("onesr", [1, 128], bf16,
                                           isOutput=False)
    st.ones8_h = nc.declare_dram_parameter("ones8", [128, 8], fp8,
                                           isOutput=False)
    st.sel_h = nc.declare_dram_parameter("sel", [128, 2], f32, isOutput=False)
    st.seln_h = nc.declare_dram_parameter("seln", [128, 2], f32, isOutput=False)
    st.y_h = nc.declare_dram_parameter("y", [TH, C], f32, isOutput=True)
    if _CACHE.get("debug"):
        st.drbi_h = nc.declare_dram_parameter("dbg_rbi", [64, 512], f32, isOutput=True)
        st.dasb_h = nc.declare_dram_parameter("dbg_asb", [128, 512], f32, isOutput=True)
        st.datA_h = nc.declare_dram_parameter("dbg_attA", [128, 1024], f32, isOutput=True)
        st.dx2_h = nc.declare_dram_parameter("dbg_x2", [128, C], f32, isOutput=True)

    st.rs_in = [nc.dram_tensor(f"rs_in{p}", [2, 128, 4, 512], fp8)
                for p in range(2)]
    st.rs_out = [nc.dram_tensor(f"rs_out{p}", [128, 4, 512], fp8)
                 for p in range(2)]

    with tile.TileContext(nc) as tc, ExitStack() as stack:
        st.tc, st.stack = tc, stack
        cst = stack.enter_context(tc.tile_pool(name="const", bufs=1))
        st.work = stack.enter_context(tc.tile_pool(name="work", bufs=4))
        st.ptp = stack.enter_context(tc.tile_pool(name="ptp", bufs=8))

        st.tri = cst.tile([128, 128], fp8, name="tri")
        st.onesr = cst.tile([1, 128], bf16, name="onesr")
        st.ones8 = cst.tile([128, 8], fp8, name="ones8")
        st.bq_sb = cst.tile([128, 4], f32, name="bq_sb")
        st.bk_sb = cst.tile([128, 4], f32, name="bk_sb")
        st.bv_sb = cst.tile([1, H], bf16, name="bv_sb")
        st.b1_sb = cst.tile([128, 32], f32, name="b1_sb")
        st.sel_sb = cst.tile([128, 2], f32, name="sel_sb")
        st.seln_sb = cst.tile([128, 2], f32, name="seln_sb")
        st.b2_sb = cst.tile([1, C], bf16, name="b2_sb")

        _front_setup(nc, st)
        for j in range(4):
            _front_qkv_j(nc, st, j)
            _front_att_j(nc, st, j)
            _front_prefetch_j(nc, st, j)
        _front_finish(nc, st)
        _phase_proj(nc, st)
        _open_ffn_pools(nc, st)
        _phase_ffn_half(nc, st, 0)
        _phase_ffn_half(nc, st, 1)
        _close_ffn_pools(nc, st)

    nc.compile()
    _CACHE[key] = nc
    return nc


def make_inputs(x, Wq, Wk, Wv, Wo, bo, W1, b1, W2, b2,
                ln1_g, ln1_b, ln2_g, ln2_b):
    """Build per-core input maps (host-side sharding + LN folding)."""
    x = np.asarray(x, np.float32)
    scale = float(C) ** -0.5

    wq_eff = ln1_g[:, None] * Wq
    wk_eff = ln1_g[:, None] * Wk * scale
    wv_eff = ln1_g[:, None] * Wv
    bq_full = ln1_b @ Wq
    bk_full = (ln1_b @ Wk) * scale
    bv_full = ln1_b @ Wv
    w1_eff = ln2_g[:, None] * W1
    b1_eff = b1 + ln2_b @ W1

    BF = ml_dtypes.bfloat16
    tri = np.triu(np.ones((128, 128), E4))
    onesr = np.ones((1, 128), BF)
    ones8 = np.ones((128, 8), E4)

    # w1 relayout: w1r[f, p, k*128 + c] = w1_eff[k*128 + p, f*128 + c]
    # scaled by W1S, split hi/lo e4m3: w1_h [32, 2, 128, 1024]
    w1r_f = np.ascontiguousarray(
        (W1S * w1_eff).astype(np.float32).reshape(8, 128, 32, 128)
        .transpose(2, 1, 0, 3).reshape(32, 128, 1024))
    w1hi = w1r_f.astype(E4)
    w1lo = (w1r_f - w1hi.astype(np.float32)).astype(E4)
    w1hl = np.ascontiguousarray(np.stack([w1hi, w1lo], axis=1))

    # w2 scaled by W2S, hi/lo: w2_h [2, F, C]
    w2s = (W2S * np.asarray(W2, np.float32))
    w2hi = w2s.astype(E4)
    w2lo = (w2s - w2hi.astype(np.float32)).astype(E4)
    w2hl = np.ascontiguousarray(np.stack([w2hi, w2lo], axis=0))

    qkv_bias = bool(np.abs(bq_full).max() or np.abs(bk_full).max()
                    or np.abs(bv_full).max())
    _CACHE["qkv_bias"] = qkv_bias
    vsc = 1.0
    in_maps = []
    for core in range(8):
        b, s = core // 2, core % 2
        cs = slice(s * H, (s + 1) * H)
        ts = slice(s * TH, (s + 1) * TH)
        own = np.arange(s * H, (s + 1) * H)
        other = np.arange((1 - s) * H, (2 - s) * H)
        perm = np.concatenate([own, other])
        in_maps.append({
            "x": np.ascontiguousarray(x[b]),
            "xres": np.ascontiguousarray(x[b, ts, :] + bo[None, :]),
            "wq": np.ascontiguousarray((W1S * wq_eff[:, cs]).astype(E4)),
            "wk": np.ascontiguousarray((W1S * wk_eff[:, cs]).astype(E4)),
            "wv": np.ascontiguousarray((W1S * wv_eff[:, cs]).astype(E4)),
            "bq": np.ascontiguousarray(bq_full[cs].reshape(4, 128).T),
            "bk": np.ascontiguousarray(bk_full[cs].reshape(4, 128).T),
            "bv": np.ascontiguousarray(
                (W1S * bv_full[cs]).reshape(1, H).astype(BF)),
            "wo": np.ascontiguousarray((W1S * Wo[perm, :]).astype(E4)),
            "w1": w1hl,
            "b1": np.ascontiguousarray((W1S * b1_eff).reshape(32, 128).T),
            "w2": w2hl,
            "b2": np.ascontiguousarray((b2 / FFS).reshape(1, C).astype(BF)),
            "tri": tri, "onesr": onesr, "ones8": ones8,
            "sel": np.tile(vsc * np.eye(2, dtype=np.float32)[s][None, :],
                           (128, 1)),
            "seln": np.tile(vsc * np.eye(2, dtype=np.float32)[1 - s][None, :],
                            (128, 1)),
        })
    return in_maps


def kernel(**inputs):
    in_maps = make_inputs(**{k: np.asarray(v, np.float32) for k, v in inputs.items()})
    nc = build_program(qkv_bias=_CACHE.get("qkv_bias", False))
    res = run_bass_kernel_spmd(nc, in_maps, list(range(8)))
    out = np.empty((B, T, C), np.float32)
    for core in range(8):
        b, s = core // 2, core % 2
        out[b, s * TH:(s + 1) * TH, :] = res.results[core]["y"]
    return out
